# revision 1
# baseline (speedup 1.0000x reference)
"""Trainium2 Bass kernel for nn_CoordinateLinear: out = W @ x + b.

Full shapes: x [1024, 16384] f32, W [1024, 1024] f32, b [1024] f32,
out [1024, 16384] f32.

Sharding: data-parallel on the batch axis — each of the 8 cores computes
out[:, c*2048:(c+1)*2048] = W @ x[:, c*2048:(c+1)*2048] + b with W and b
replicated.

Per-core device kernel: K=1024 contraction in 8 tiles of 128 partitions,
M=1024 output rows in 8 tiles of 128, N=2048 batch in 4 slabs of 512.
W^T is resident in SBUF; x streams per iteration in 8 contiguous
[128, 2048] per-k transfers (double buffered); loop order m -> k -> n
keeps 4 PSUM banks accumulating in parallel with 4 consecutive matmuls
sharing the same stationary weight tile; the PSUM->SBUF eviction fuses
the bias add on the scalar engine; output leaves in one [128, 2048]
DMA per m-group.

MODE selects the matmul numerics/throughput tradeoff:
  "f32"    exact fp32 matmul, 4 PE cycles/row.
  "f32r"   single-pass fp32r (fp32 rounded to 11-bit mantissa on host),
           1 PE cycle/row, ~1e-4 relative error.
  "bf16"   single-pass bf16, f32 output. 16-bit moving operand streams
           2 elem/cycle through the PE -> ~2x over f32r; ~2.5e-3 error.
  "bf16o"  bf16 in AND out (host upconverts the result); halves the
           output DMA as well; ~3.8e-3 error.
  "split3" bf16 hi/lo split, out = Wh@xh + Wh@xl + Wl@xh, 3 passes,
           fp32-quality error.
"""

import contextlib
import os
import sys

if "/opt/trn_rl_repo" not in sys.path:
    sys.path.insert(0, "/opt/trn_rl_repo")

import ml_dtypes
import numpy as np

import concourse.bass as bass
import concourse.mybir as mybir
import concourse.tile as tile
from concourse import bacc
from concourse.bass_utils import run_bass_kernel_spmd

N_CORES = 8
P = 128
K = 1024
M = 1024
N_FULL = 16384
N_CORE = N_FULL // N_CORES  # 2048
N_TILE = 512
K_T = K // P  # 8
M_T = M // P  # 8
N_T = N_CORE // N_TILE  # 4

# Graded default: f32ro — fp32r matmul (at the PE roofline, 1 cycle/row)
# with bf16 output eviction, which halves the output DMA (16.8 -> 12.6
# MiB/iter total). Equal to f32r when HBM is quiet (DMA fully hidden under
# compute); measurably tighter worst case when the HBM stack is contended
# (interleaved race: med 72.6 vs 74.0 us, max 78 vs 108 us). Rel err
# 2.6e-3 on HW vs the 2e-2 gate.
MODE = os.environ.get("KMODE", "f32ro")

BF16 = ml_dtypes.bfloat16
_compiled = {}


def _round_fp32r(a):
    """Round fp32 to fp32r (1s + 8e + 11m in the top 20 bits), RNE."""
    u = np.ascontiguousarray(a, dtype=np.float32).view(np.uint32)
    r = (u + np.uint32(0x7FF) + ((u >> np.uint32(12)) & np.uint32(1))) & np.uint32(
        0xFFFFF000
    )
    return r.view(np.float32)


def _mode_cfg(mode):
    f32 = mybir.dt.float32
    bf16 = mybir.dt.bfloat16
    # (in_dt, out_dt, w_names, x_names, terms)
    if mode == "f32":
        return f32, f32, ["wT"], ["x"], [("wT", "x")]
    if mode == "f32r":
        return mybir.dt.float32r, f32, ["wT"], ["x"], [("wT", "x")]
    if mode == "f32ro":
        # fp32r matmul, bf16 output eviction (host upconverts): halves the
        # out DMA for robustness when HBM bandwidth is contended.
        return mybir.dt.float32r, bf16, ["wT"], ["x"], [("wT", "x")]
    if mode == "bf16":
        return bf16, f32, ["wT"], ["x"], [("wT", "x")]
    if mode == "bf16o":
        return bf16, bf16, ["wT"], ["x"], [("wT", "x")]
    if mode == "split3":
        return bf16, f32, ["wT_hi", "wT_lo"], ["x_hi", "x_lo"], [
            ("wT_hi", "x_hi"),
            ("wT_hi", "x_lo"),
            ("wT_lo", "x_hi"),
        ]
    raise ValueError(mode)


def _build(mode, repeat=1, bench_internal=False, evict="act", probe=None):
    # probe (bench-only): "noout" drops the output DMAs; "nox" drops the x
    # slab loads (matmuls read stale slabs); isolates the steady-state limiter.
    nc = bacc.Bacc("TRN2", target_bir_lowering=False, debug=False)

    f32 = mybir.dt.float32
    in_dt, out_dt, w_names, x_names, terms = _mode_cfg(mode)
    itemsize = 2 if in_dt == mybir.dt.bfloat16 else 4

    io_kind = "Internal" if bench_internal else None
    w_kind = io_kind or "ExternalInput"
    w_d = {nm: nc.dram_tensor(nm, [K, M], in_dt, kind=w_kind) for nm in w_names}
    x_d = {nm: nc.dram_tensor(nm, [K, N_CORE], in_dt, kind=w_kind) for nm in x_names}
    b_d = nc.dram_tensor("bias", [P, M_T], f32, kind=w_kind)
    o_d = nc.dram_tensor("out", [M, N_CORE], out_dt, kind=io_kind or "ExternalOutput")
    if bench_internal:
        tok_i = nc.dram_tensor("tok_i", [P, 16], f32, kind="ExternalInput")
        tok_o = nc.dram_tensor("tok_o", [P, 16], f32, kind="ExternalOutput")

    # SBUF budget per partition (~208 KiB usable): resident W is
    # K_T*M*itemsize, each x buffer K_T*N_CORE*itemsize; pick x double/
    # triple buffering to fit.
    w_bytes = len(w_names) * K_T * M * itemsize
    x_bytes = len(x_names) * K_T * N_CORE * itemsize
    x_bufs = max(2, min(4, (190 * 1024 - w_bytes) // max(x_bytes, 1)))
    with tile.TileContext(nc) as tc:
        with (
            tc.tile_pool(name="wpool", bufs=1) as wpool,
            tc.tile_pool(name="xpool", bufs=x_bufs) as xpool,
            tc.tile_pool(name="bpool", bufs=1) as bpool,
            tc.tile_pool(name="opool", bufs=3) as opool,
            tc.tile_pool(name="pspool", bufs=8, space="PSUM") as pspool,
        ):
            if bench_internal:
                tok_sb = bpool.tile([P, 16], f32, tag="tok")
                nc.sync.dma_start(out=tok_sb[:], in_=tok_i[:])
                nc.sync.dma_start(out=tok_o[:], in_=tok_sb[:])

            # repeat == 0 builds a "null" benchmark NEFF: token roundtrip only.
            bias_sb = None
            w_sb = {}
            if repeat > 0:
                bias_sb = bpool.tile([P, M_T], f32, tag="bias")
                nc.sync.dma_start(out=bias_sb[:], in_=b_d[:])

                # Resident weights: w_sb[nm] is [P, K_T * M]; k-tile k lives
                # at free-dim offset k*M. One contiguous [128, M] transfer
                # per k.
                for nm in w_names:
                    t = wpool.tile([P, K_T * M], in_dt, tag=f"w_{nm}", name=f"{nm}_sb")
                    for k in range(K_T):
                        nc.sync.dma_start(
                            out=t[:, k * M : (k + 1) * M],
                            in_=w_d[nm][k * P : (k + 1) * P, :],
                        )
                    w_sb[nm] = t

            n_mm = len(terms) * K_T
            loop_cm = contextlib.nullcontext()
            if probe and probe.startswith("hwl") and repeat > 0:
                # Hardware-loop bench: wrap the repeat-unrolled body in a
                # For_i with a static trip count. Two NEFFs with different
                # trip counts give an overhead-free device-time differential.
                trip = int(probe[3:])
                loop_cm = tc.For_i(0, trip)
            with loop_cm:
                _emit_body(
                    nc, tc, repeat, x_names, xpool, pspool, opool, in_dt, out_dt,
                    x_d, o_d, w_sb, bias_sb, terms, n_mm, evict, probe,
                )

    nc.compile()
    return nc


def _emit_body(
    nc, tc, repeat, x_names, xpool, pspool, opool, in_dt, out_dt,
    x_d, o_d, w_sb, bias_sb, terms, n_mm, evict, probe,
):
    f32 = mybir.dt.float32
    for r in range(repeat):
        # Stream this iteration's x: [P, K_T * N_CORE] per tensor,
        # k-tile k at free-dim offset k*N_CORE, one contiguous
        # [128, 2048] transfer per k.
        x_sb = {}
        for nm in x_names:
            x_sb[nm] = xpool.tile(
                [P, K_T * N_CORE], in_dt, tag=f"x_{nm}", name=f"{nm}_sl_{r}"
            )
        k_load = 1 if probe == "nox" else K_T
        for k in range(k_load):
            for nm in x_names:
                nc.sync.dma_start(
                    out=x_sb[nm][:, k * N_CORE : (k + 1) * N_CORE],
                    in_=x_d[nm][k * P : (k + 1) * P, :],
                )

        # Loop order m -> k -> n: 4 PSUM banks accumulate in
        # parallel and 4 consecutive matmuls share the same
        # stationary weight tile.
        for m in range(M_T):
            pss = [
                pspool.tile([P, N_TILE], f32, tag="ps", name=f"ps_{r}_{m}_{n}")
                for n in range(N_T)
            ]
            o_sb = opool.tile([P, N_CORE], out_dt, tag="o", name=f"o_{r}_{m}")
            i = 0
            for wn, xn in terms:
                for k in range(K_T):
                    for n in range(N_T):
                        nc.tensor.matmul(
                            pss[n][:],
                            w_sb[wn][:, k * M + m * P : k * M + (m + 1) * P],
                            x_sb[xn][
                                :,
                                k * N_CORE + n * N_TILE : k * N_CORE
                                + (n + 1) * N_TILE,
                            ],
                            start=(i == 0),
                            stop=(i == n_mm - 1),
                        )
                    i += 1
            for n in range(N_T):
                if evict == "act":
                    nc.scalar.activation(
                        o_sb[:, n * N_TILE : (n + 1) * N_TILE],
                        pss[n][:],
                        mybir.ActivationFunctionType.Identity,
                        bias=bias_sb[:, m : m + 1],
                    )
                else:
                    nc.vector.tensor_scalar_add(
                        o_sb[:, n * N_TILE : (n + 1) * N_TILE],
                        pss[n][:],
                        bias_sb[:, m : m + 1],
                    )
            if probe != "noout":
                nc.sync.dma_start(
                    out=o_d[m * P : (m + 1) * P, :],
                    in_=o_sb[:],
                )


def _get_compiled(mode, repeat=1, bench_internal=False, evict="act", probe=None):
    key = (mode, repeat, bench_internal, evict, probe)
    if key not in _compiled:
        _compiled[key] = _build(mode, repeat, bench_internal, evict, probe)
    return _compiled[key]


def _in_maps(mode, x, weight, bias):
    x = np.asarray(x, dtype=np.float32)
    wT = np.ascontiguousarray(np.asarray(weight, dtype=np.float32).T)
    b_pre = np.ascontiguousarray(np.asarray(bias, dtype=np.float32).reshape(M_T, P).T)

    if mode == "f32":
        w_parts = {"wT": wT}
        x_full = {"x": x}
    elif mode in ("f32r", "f32ro"):
        w_parts = {"wT": _round_fp32r(wT)}
        x_full = {"x": _round_fp32r(x)}
    elif mode in ("bf16", "bf16o"):
        w_parts = {"wT": wT.astype(BF16)}
        x_full = {"x": x.astype(BF16)}
    elif mode == "split3":
        wh = wT.astype(BF16)
        wl = (wT - wh.astype(np.float32)).astype(BF16)
        xh = x.astype(BF16)
        xl = (x - xh.astype(np.float32)).astype(BF16)
        w_parts = {"wT_hi": wh, "wT_lo": wl}
        x_full = {"x_hi": xh, "x_lo": xl}
    else:
        raise ValueError(mode)

    maps = []
    for c in range(N_CORES):
        m = dict(w_parts)
        m["bias"] = b_pre
        for nm, arr in x_full.items():
            m[nm] = np.ascontiguousarray(arr[:, c * N_CORE : (c + 1) * N_CORE])
        maps.append(m)
    return maps


LOOP = "wreuse"  # kept for the test harness API; the m->k->n loop is default


def kernel(x, weight, bias):
    nc = _get_compiled(MODE, probe=LOOP)
    maps = _in_maps(MODE, x, weight, bias)
    last_err = None
    for _ in range(3):
        try:
            res = run_bass_kernel_spmd(nc, maps, core_ids=list(range(N_CORES)))
            break
        except Exception as exc:  # transient NRT device errors; retry
            last_err = exc
    else:
        raise last_err
    out = np.concatenate([res.results[c]["out"] for c in range(N_CORES)], axis=1)
    return np.ascontiguousarray(out.astype(np.float32))



# revision 11
# speedup vs baseline: 2.0678x; 2.0678x over previous
"""Trainium2 Bass kernel for nn_CoordinateLinear: out = W @ x + b.

Full shapes: x [1024, 16384] f32, W [1024, 1024] f32, b [1024] f32,
out [1024, 16384] f32.

Sharding: data-parallel on the batch axis — each of the 8 cores computes
out[:, c*2048:(c+1)*2048] = W @ x[:, c*2048:(c+1)*2048] + b with W and b
replicated.

Per-core device kernel: K=1024 contraction in 8 tiles of 128 partitions,
M=1024 output rows in 8 tiles of 128, N=2048 batch in 4 slabs of 512.
W^T is resident in SBUF; x streams per iteration in 8 contiguous
[128, 2048] per-k transfers (double buffered); loop order m -> k -> n
keeps 4 PSUM banks accumulating in parallel with 4 consecutive matmuls
sharing the same stationary weight tile; the PSUM->SBUF eviction fuses
the bias add on the scalar engine; output leaves in one [128, 2048]
DMA per m-group.

MODE selects the matmul numerics/throughput tradeoff:
  "f32"    exact fp32 matmul, 4 PE cycles/row.
  "f32r"   single-pass fp32r (fp32 rounded to 11-bit mantissa on host),
           1 PE cycle/row, ~1e-4 relative error.
  "bf16"   single-pass bf16, f32 output. 16-bit moving operand streams
           2 elem/cycle through the PE -> ~2x over f32r; ~2.5e-3 error.
  "bf16o"  bf16 in AND out (host upconverts the result); halves the
           output DMA as well; ~3.8e-3 error.
  "split3" bf16 hi/lo split, out = Wh@xh + Wh@xl + Wl@xh, 3 passes,
           fp32-quality error.
"""

import contextlib
import os
import sys

if "/opt/trn_rl_repo" not in sys.path:
    sys.path.insert(0, "/opt/trn_rl_repo")

import ml_dtypes
import numpy as np

import concourse.bass as bass
import concourse.mybir as mybir
import concourse.tile as tile
from concourse import bacc
from concourse.bass_utils import run_bass_kernel_spmd

N_CORES = 8
P = 128
K = 1024
M = 1024
N_FULL = 16384
N_CORE = N_FULL // N_CORES  # 2048
N_TILE = 512
K_T = K // P  # 8
M_T = M // P  # 8
N_T = N_CORE // N_TILE  # 4

# Graded default: bf16o — bf16 matmul with bf16 output eviction.
# bf16 and f32r both stream 1 moving row/cycle through the PE, but every
# InstMatmult self-loads its stationary tile, serializing the load with
# the stream: 128 cycles for f32r vs 64 for bf16 (fast-weight-load reads
# 2 bf16/cycle). Measured on HW: f32ro 256x(128+512)cyc = 68.3us model,
# bf16o 256x(64+512) = 61.4us model; both match. bf16o also halves the
# x DMA (12.6 -> 8.4 MiB/iter total), tightening the contended case.
# Rel err 3.3e-3 on HW vs the 2e-2 gate.
MODE = os.environ.get("KMODE", "bf16o")

BF16 = ml_dtypes.bfloat16
_compiled = {}


def _round_fp32r(a):
    """Round fp32 to fp32r (1s + 8e + 11m in the top 20 bits), RNE."""
    u = np.ascontiguousarray(a, dtype=np.float32).view(np.uint32)
    r = (u + np.uint32(0x7FF) + ((u >> np.uint32(12)) & np.uint32(1))) & np.uint32(
        0xFFFFF000
    )
    return r.view(np.float32)


def _mode_cfg(mode):
    f32 = mybir.dt.float32
    bf16 = mybir.dt.bfloat16
    # (in_dt, out_dt, w_names, x_names, terms)
    if mode == "f32":
        return f32, f32, ["wT"], ["x"], [("wT", "x")]
    if mode == "f32r":
        return mybir.dt.float32r, f32, ["wT"], ["x"], [("wT", "x")]
    if mode == "f32ro":
        # fp32r matmul, bf16 output eviction (host upconverts): halves the
        # out DMA for robustness when HBM bandwidth is contended.
        return mybir.dt.float32r, bf16, ["wT"], ["x"], [("wT", "x")]
    if mode == "bf16":
        return bf16, f32, ["wT"], ["x"], [("wT", "x")]
    if mode == "bf16o":
        return bf16, bf16, ["wT"], ["x"], [("wT", "x")]
    if mode == "split3":
        return bf16, f32, ["wT_hi", "wT_lo"], ["x_hi", "x_lo"], [
            ("wT_hi", "x_hi"),
            ("wT_hi", "x_lo"),
            ("wT_lo", "x_hi"),
        ]
    raise ValueError(mode)


def _parse_probe(probe):
    """probe is a '+'-separated flag string (bench-only):
      noout   drop the output DMAs
      nox     drop the x slab loads (matmuls read stale slabs)
      w1      all matmuls use the same stationary tile (tests whether
              codegen elides redundant weight reloads)
      hwlNN   wrap the repeat-unrolled body in a For_i with trip NN
    """
    flags = set()
    trip = None
    for p in (probe or "").split("+"):
        p = p.strip()
        if not p:
            continue
        if p.startswith("hwl"):
            trip = int(p[3:])
        else:
            flags.add(p)
    return flags, trip


def _build(mode, repeat=1, bench_internal=False, evict="act", probe=None):
    nc = bacc.Bacc("TRN2", target_bir_lowering=False, debug=False)

    f32 = mybir.dt.float32
    in_dt, out_dt, w_names, x_names, terms = _mode_cfg(mode)
    itemsize = 2 if in_dt == mybir.dt.bfloat16 else 4

    io_kind = "Internal" if bench_internal else None
    w_kind = io_kind or "ExternalInput"
    w_d = {nm: nc.dram_tensor(nm, [K, M], in_dt, kind=w_kind) for nm in w_names}
    x_d = {nm: nc.dram_tensor(nm, [K, N_CORE], in_dt, kind=w_kind) for nm in x_names}
    b_d = nc.dram_tensor("bias", [P, M_T], f32, kind=w_kind)
    o_d = nc.dram_tensor("out", [M, N_CORE], out_dt, kind=io_kind or "ExternalOutput")
    if bench_internal:
        tok_i = nc.dram_tensor("tok_i", [P, 16], f32, kind="ExternalInput")
        tok_o = nc.dram_tensor("tok_o", [P, 16], f32, kind="ExternalOutput")

    # SBUF budget per partition (~208 KiB usable): resident W is
    # K_T*M*itemsize, each x buffer K_T*N_CORE*itemsize; pick x double/
    # triple buffering to fit.
    w_bytes = len(w_names) * K_T * M * itemsize
    x_bytes = len(x_names) * K_T * N_CORE * itemsize
    x_bufs = max(2, min(4, (190 * 1024 - w_bytes) // max(x_bytes, 1)))
    with tile.TileContext(nc) as tc:
        with (
            tc.tile_pool(name="wpool", bufs=1) as wpool,
            tc.tile_pool(name="xpool", bufs=x_bufs) as xpool,
            tc.tile_pool(name="bpool", bufs=1) as bpool,
            tc.tile_pool(name="opool", bufs=3) as opool,
            tc.tile_pool(name="pspool", bufs=8, space="PSUM") as pspool,
        ):
            if bench_internal:
                tok_sb = bpool.tile([P, 16], f32, tag="tok")
                nc.sync.dma_start(out=tok_sb[:], in_=tok_i[:])
                nc.sync.dma_start(out=tok_o[:], in_=tok_sb[:])

            # repeat == 0 builds a "null" benchmark NEFF: token roundtrip only.
            bias_sb = None
            w_sb = {}
            if repeat > 0:
                bias_sb = bpool.tile([P, M_T], f32, tag="bias")
                nc.sync.dma_start(out=bias_sb[:], in_=b_d[:])

                # Resident weights: w_sb[nm] is [P, K_T * M]; k-tile k lives
                # at free-dim offset k*M. One contiguous [128, M] transfer
                # per k.
                for nm in w_names:
                    t = wpool.tile([P, K_T * M], in_dt, tag=f"w_{nm}", name=f"{nm}_sb")
                    for k in range(K_T):
                        nc.sync.dma_start(
                            out=t[:, k * M : (k + 1) * M],
                            in_=w_d[nm][k * P : (k + 1) * P, :],
                        )
                    w_sb[nm] = t

            n_mm = len(terms) * K_T
            flags, trip = _parse_probe(probe)
            loop_cm = contextlib.nullcontext()
            if trip is not None and repeat > 0:
                # Hardware-loop bench: wrap the repeat-unrolled body in a
                # For_i with a static trip count. Two NEFFs with different
                # trip counts give an overhead-free device-time differential.
                loop_cm = tc.For_i(0, trip)
            with loop_cm:
                _emit_body(
                    nc, tc, repeat, x_names, xpool, pspool, opool, in_dt, out_dt,
                    x_d, o_d, w_sb, bias_sb, terms, n_mm, evict, flags,
                )

    nc.compile()
    return nc


def _emit_body(
    nc, tc, repeat, x_names, xpool, pspool, opool, in_dt, out_dt,
    x_d, o_d, w_sb, bias_sb, terms, n_mm, evict, flags,
):
    f32 = mybir.dt.float32
    for r in range(repeat):
        # Stream this iteration's x: [P, K_T * N_CORE] per tensor,
        # k-tile k at free-dim offset k*N_CORE, one contiguous
        # [128, 2048] transfer per k.
        x_sb = {}
        for nm in x_names:
            x_sb[nm] = xpool.tile(
                [P, K_T * N_CORE], in_dt, tag=f"x_{nm}", name=f"{nm}_sl_{r}"
            )
        k_load = 1 if "nox" in flags else K_T
        for k in range(k_load):
            for nm in x_names:
                nc.sync.dma_start(
                    out=x_sb[nm][:, k * N_CORE : (k + 1) * N_CORE],
                    in_=x_d[nm][k * P : (k + 1) * P, :],
                )

        # Loop order m -> k -> n: 4 PSUM banks accumulate in
        # parallel and 4 consecutive matmuls share the same
        # stationary weight tile.
        for m in range(M_T):
            pss = [
                pspool.tile([P, N_TILE], f32, tag="ps", name=f"ps_{r}_{m}_{n}")
                for n in range(N_T)
            ]
            o_sb = opool.tile([P, N_CORE], out_dt, tag="o", name=f"o_{r}_{m}")
            # Weight-load policy: an InstMatmult self-loads its stationary
            # operand, serializing ~64 cycles (bf16 FWL) with each 512-cycle
            # stream. A standalone InstLdweights immediately before a
            # matmul makes walrus emit that matmul non-self-loading, and
            # the PE reorder window pulls the load into the background
            # weight buffer under the previous matmul's streaming. Only
            # legal for 16-bit dtypes (f32/f32r must self-load).
            # Measured verdict (interleaved + min-slope batches): extra
            # standalone ldweights SERIALIZE (no reorder-window hiding) —
            # lwall/lw2 cost +8..13us; per-group lw is noise-neutral.
            # Default: plain self-loading matmuls.
            is16 = in_dt in (mybir.dt.bfloat16, mybir.dt.float16)
            lw_all = "lwall" in flags
            lw2 = "lw2" in flags
            lw_group = "lw" in flags
            i = 0
            for wn, xn in terms:
                for k in range(K_T):
                    w_off = 0 if "w1" in flags else k * M + m * P
                    w_ap = w_sb[wn][:, w_off : w_off + P]
                    if lw_group and is16:
                        nc.tensor.ldweights(w_ap)
                    for n in range(N_T):
                        if is16 and (lw_all or (lw2 and n % 2 == 0)):
                            nc.tensor.ldweights(w_ap)
                        nc.tensor.matmul(
                            pss[n][:],
                            w_ap,
                            x_sb[xn][
                                :,
                                k * N_CORE + n * N_TILE : k * N_CORE
                                + (n + 1) * N_TILE,
                            ],
                            start=(i == 0),
                            stop=(i == n_mm - 1),
                        )
                    i += 1
            for n in range(N_T):
                if evict == "act":
                    nc.scalar.activation(
                        o_sb[:, n * N_TILE : (n + 1) * N_TILE],
                        pss[n][:],
                        mybir.ActivationFunctionType.Identity,
                        bias=bias_sb[:, m : m + 1],
                    )
                else:
                    nc.vector.tensor_scalar_add(
                        o_sb[:, n * N_TILE : (n + 1) * N_TILE],
                        pss[n][:],
                        bias_sb[:, m : m + 1],
                    )
            if "noout" not in flags:
                nc.sync.dma_start(
                    out=o_d[m * P : (m + 1) * P, :],
                    in_=o_sb[:],
                )


def _get_compiled(mode, repeat=1, bench_internal=False, evict="act", probe=None):
    key = (mode, repeat, bench_internal, evict, probe)
    if key not in _compiled:
        _compiled[key] = _build(mode, repeat, bench_internal, evict, probe)
    return _compiled[key]


def _in_maps(mode, x, weight, bias):
    x = np.asarray(x, dtype=np.float32)
    wT = np.ascontiguousarray(np.asarray(weight, dtype=np.float32).T)
    b_pre = np.ascontiguousarray(np.asarray(bias, dtype=np.float32).reshape(M_T, P).T)

    if mode == "f32":
        w_parts = {"wT": wT}
        x_full = {"x": x}
    elif mode in ("f32r", "f32ro"):
        w_parts = {"wT": _round_fp32r(wT)}
        x_full = {"x": _round_fp32r(x)}
    elif mode in ("bf16", "bf16o"):
        w_parts = {"wT": wT.astype(BF16)}
        x_full = {"x": x.astype(BF16)}
    elif mode == "split3":
        wh = wT.astype(BF16)
        wl = (wT - wh.astype(np.float32)).astype(BF16)
        xh = x.astype(BF16)
        xl = (x - xh.astype(np.float32)).astype(BF16)
        w_parts = {"wT_hi": wh, "wT_lo": wl}
        x_full = {"x_hi": xh, "x_lo": xl}
    else:
        raise ValueError(mode)

    maps = []
    for c in range(N_CORES):
        m = dict(w_parts)
        m["bias"] = b_pre
        for nm, arr in x_full.items():
            m[nm] = np.ascontiguousarray(arr[:, c * N_CORE : (c + 1) * N_CORE])
        maps.append(m)
    return maps


LOOP = None  # kept for the test harness API; the m->k->n loop is default


def kernel(x, weight, bias):
    nc = _get_compiled(MODE, probe=LOOP)
    maps = _in_maps(MODE, x, weight, bias)
    last_err = None
    for _ in range(3):
        try:
            res = run_bass_kernel_spmd(nc, maps, core_ids=list(range(N_CORES)))
            break
        except Exception as exc:  # transient NRT device errors; retry
            last_err = exc
    else:
        raise last_err
    out = np.concatenate([res.results[c]["out"] for c in range(N_CORES)], axis=1)
    return np.ascontiguousarray(out.astype(np.float32))

